# revision 6
# baseline (speedup 1.0000x reference)
"""LoRA BF16 Linear kernel for 8x Trainium2 NeuronCores.

Computes out = x @ W^T + b + 2.0 * (x @ A^T) @ B^T
  x [4,4096,4096] f32, W [4096,4096] f32, b [4096] f32, A [16,4096], B [4096,16]

Strategy: data-parallel over the 16384 tokens (2048 per core). Each core:
  - holds x_shard^T (bf16, [4096, 2048]) resident in SBUF
  - streams W^T (bf16) in 256-wide output-column slabs
  - computes out[tokens, dout] in PSUM via 32 K=128 bf16 matmuls
  - LoRA branch + bias are folded in as one extra augmented matmul per tile:
      rows 0..15 = (2A @ x^T) (computed on-device), row 16 = ones;
      W-side rows 0..15 = B^T, row 16 = b.
No collectives needed; host shards inputs and concatenates core outputs.
"""

import numpy as np
import ml_dtypes
from contextlib import ExitStack

BF16 = ml_dtypes.bfloat16

# Problem shapes (hardcoded per harness contract)
B_, S, D_IN, D_OUT, R = 4, 4096, 4096, 4096, 16
N_CORES = 8
TOK = B_ * S                 # 16384 tokens total
T = TOK // N_CORES           # 2048 tokens per core
KO = D_IN // 128             # 32 k-tiles
NT = 256                     # output-column tile width
N_TILES = D_OUT // NT        # 16
M_TILES = T // 128           # 16
SCALING = 32.0 / 16.0

_CACHE: dict = {}
_ONES = np.ones((1, T), dtype=BF16)


def _build_bass():
    import concourse.bacc as bacc
    import concourse.mybir as mybir
    import concourse.tile as tile
    from concourse.bass import ts

    nc = bacc.Bacc("TRN2", target_bir_lowering=False, debug=False)
    BF = mybir.dt.bfloat16
    F32 = mybir.dt.float32

    xT = nc.dram_tensor("xT", [D_IN, T], BF, kind="ExternalInput")
    WT = nc.dram_tensor("WT", [D_IN, D_OUT], BF, kind="ExternalInput")
    ATp = nc.dram_tensor("ATp", [128, KO * R], BF, kind="ExternalInput")
    WBaug = nc.dram_tensor("WBaug", [128, D_OUT], BF, kind="ExternalInput")
    ones = nc.dram_tensor("ones", [1, T], BF, kind="ExternalInput")
    out = nc.dram_tensor("out", [T, D_OUT], F32, kind="ExternalOutput")

    xT_r = xT.ap().rearrange("(ko p) t -> p ko t", p=128)
    WT_r = WT.ap().rearrange("(ko p) o -> p ko o", p=128)
    out_ap = out.ap()

    with tile.TileContext(nc) as tc:
        with ExitStack() as ctx:
            resident = ctx.enter_context(tc.tile_pool(name="resident", bufs=1))
            wtpool = ctx.enter_context(tc.tile_pool(name="wtpool", bufs=2))
            opool = ctx.enter_context(tc.tile_pool(name="opool", bufs=4))
            pspool = ctx.enter_context(
                tc.tile_pool(name="pspool", bufs=4, space="PSUM")
            )

            # Resident x^T: [128, 32, 2048] bf16 = 128KB/partition
            xT_sb = resident.tile([128, KO, T], BF)
            for ko in range(KO):
                nc.sync.dma_start(out=xT_sb[:, ko, :], in_=xT_r[:, ko, :])

            # 2*A^T packed [128, ko*r], B^T+bias augmented [128, dout]
            AT_sb = resident.tile([128, KO * R], BF)
            nc.sync.dma_start(out=AT_sb, in_=ATp.ap())
            AT_r = AT_sb.rearrange("p (ko r) -> p ko r", r=R)
            WB_sb = resident.tile([128, D_OUT], BF)
            nc.sync.dma_start(out=WB_sb, in_=WBaug.ap())

            # xa^T augmented: rows 0-15 = 2*A@x^T, row 16 = ones, rest zero
            xaT_sb = resident.tile([128, T], BF)
            nc.any.memset(xaT_sb, 0.0)
            nc.sync.dma_start(out=xaT_sb[R : R + 1, :], in_=ones.ap())

            # Prologue: xa^T[r, t] = sum_k (2A)^T[k, r] * x^T[k, t]
            for tw in range(T // 512):
                ps_xa = pspool.tile([16, 512], F32, tag="ps_xa", bufs=2)
                for ko in range(KO):
                    nc.tensor.matmul(
                        ps_xa,
                        AT_r[:, ko, :],
                        xT_sb[:, ko, ts(tw, 512)],
                        start=(ko == 0),
                        stop=(ko == KO - 1),
                    )
                nc.vector.tensor_copy(out=xaT_sb[0:R, ts(tw, 512)], in_=ps_xa)

            # Main: out[m-tile, n-tile] = sum_ko xT_k^T @ WT_k  (+ aug)
            for n in range(N_TILES):
                wt_sb = wtpool.tile([128, KO, NT], BF)
                nc.sync.dma_start(out=wt_sb, in_=WT_r[:, :, ts(n, NT)])
                for m in range(M_TILES):
                    ps = pspool.tile([128, NT], F32, tag="ps")
                    for ko in range(KO):
                        nc.tensor.matmul(
                            ps,
                            xT_sb[:, ko, ts(m, 128)],
                            wt_sb[:, ko, :],
                            start=(ko == 0),
                            stop=False,
                        )
                    nc.tensor.matmul(
                        ps,
                        xaT_sb[:, ts(m, 128)],
                        WB_sb[:, ts(n, NT)],
                        start=False,
                        stop=True,
                    )
                    ob = opool.tile([128, NT], F32)
                    nc.vector.tensor_copy(out=ob, in_=ps)
                    nc.scalar.dma_start(
                        out=out_ap[ts(m, 128), ts(n, NT)], in_=ob
                    )

    nc.compile()
    return nc


def _get_nc():
    if "nc" not in _CACHE:
        _CACHE["nc"] = _build_bass()
    return _CACHE["nc"]


def kernel(x, W, b, A, B):
    from concourse.bass_utils import run_bass_kernel_spmd

    nc = _get_nc()

    # Host-side layout prep (sharding + dtype/layout only)
    xf = np.ascontiguousarray(x.reshape(TOK, D_IN)).astype(BF16)
    WTh = np.ascontiguousarray(W.T).astype(BF16)            # [d_in, d_out]
    # (2A)^T packed so SBUF tile [128, ko*16] loads contiguously
    ATh = (SCALING * A).T.astype(BF16)                       # [d_in, r]
    ATp = np.ascontiguousarray(
        ATh.reshape(KO, 128, R).transpose(1, 0, 2).reshape(128, KO * R)
    )
    WBh = np.zeros((128, D_OUT), dtype=BF16)
    WBh[0:R] = B.T.astype(BF16)
    WBh[R] = b.astype(BF16)

    in_maps = []
    for c in range(N_CORES):
        xTc = np.ascontiguousarray(xf[c * T : (c + 1) * T].T)  # [d_in, T] bf16
        in_maps.append(
            {"xT": xTc, "WT": WTh, "ATp": ATp, "WBaug": WBh, "ones": _ONES}
        )

    res = run_bass_kernel_spmd(nc, in_maps, core_ids=list(range(N_CORES)))
    outs = [r["out"] for r in res.results]
    return np.concatenate(outs, axis=0).reshape(B_, S, D_OUT).astype(np.float32)


# revision 12
# speedup vs baseline: 1.7857x; 1.7857x over previous
"""LoRA BF16 Linear kernel for 8x Trainium2 NeuronCores.

Computes out = x @ W^T + b + 2.0 * (x @ A^T) @ B^T
  x [4,4096,4096] f32, W [4096,4096] f32, b [4096] f32, A [16,4096], B [4096,16]

Strategy: data-parallel over the 16384 tokens (2048 per core). Each core:
  - holds x_shard^T (bf16) resident in SBUF (in m-blocks)
  - streams W^T (bf16) in output-column slabs
  - computes out[tokens, dout] in PSUM via 32 K=128 bf16 matmuls
  - LoRA branch + bias are folded in as one extra augmented matmul per tile:
      rows 0..15 = (2A @ x^T) (computed on-device), row 16 = ones;
      W-side rows 0..15 = B^T, row 16 = b.
No collectives needed; host shards inputs and concatenates core outputs.
"""

import os
import numpy as np
import ml_dtypes
from contextlib import ExitStack

BF16 = ml_dtypes.bfloat16

# Problem shapes (hardcoded per harness contract)
B_, S, D_IN, D_OUT, R = 4, 4096, 4096, 4096, 16
N_CORES = 8
TOK = B_ * S                 # 16384 tokens total
T = TOK // N_CORES           # 2048 tokens per core
KO = D_IN // 128             # 32 k-tiles
SCALING = 32.0 / 16.0

_CACHE: dict = {}
_ONES = np.ones((1, T), dtype=BF16)

VARIANT = os.environ.get("KERNEL_VARIANT", "v1")


def _build_bass(variant=None):
    import concourse.bacc as bacc
    import concourse.mybir as mybir
    import concourse.tile as tile
    from concourse.bass import ts

    variant = variant or VARIANT
    flags = variant.split("-")
    base = flags[0]
    if base == "v0":
        NT, MB, k_stride = 256, 1, 1
    elif base == "v1":
        NT, MB, k_stride = 512, 2, 1
    elif base == "v2":  # PE-light probe: only 1 of 32 k matmuls (WRONG results)
        NT, MB, k_stride = 256, 1, 32
    else:
        raise ValueError(variant)
    NOSTORE = "nostore" in flags   # skip output DMA (wrong results)
    NOCOPY = "nocopy" in flags     # skip psum->sbuf copy too
    NOWT = "nowt" in flags         # load wt slab once, reuse (wrong results)
    PELIGHT = "pelight" in flags   # only 1 of 32 k matmuls
    if PELIGHT:
        k_stride = 32

    N_TILES = D_OUT // NT
    TB = T // MB            # tokens per block
    M_TILES = TB // 128     # m-tiles per block

    nc = bacc.Bacc("TRN2", target_bir_lowering=False, debug=False)
    BF = mybir.dt.bfloat16
    F32 = mybir.dt.float32

    xT = nc.dram_tensor("xT", [D_IN, T], BF, kind="ExternalInput")
    WT = nc.dram_tensor("WT", [D_IN, D_OUT], BF, kind="ExternalInput")
    ATp = nc.dram_tensor("ATp", [128, KO * R], BF, kind="ExternalInput")
    WBaug = nc.dram_tensor("WBaug", [128, D_OUT], BF, kind="ExternalInput")
    ones = nc.dram_tensor("ones", [1, T], BF, kind="ExternalInput")
    out = nc.dram_tensor("out", [T, D_OUT], F32, kind="ExternalOutput")

    xT_r = xT.ap().rearrange("(ko p) t -> p ko t", p=128)
    WT_r = WT.ap().rearrange("(ko p) o -> p ko o", p=128)
    out_ap = out.ap()

    with tile.TileContext(nc) as tc:
        with ExitStack() as ctx:
            resident = ctx.enter_context(tc.tile_pool(name="resident", bufs=1))
            xpool = ctx.enter_context(tc.tile_pool(name="xpool", bufs=1))
            wtpool = ctx.enter_context(tc.tile_pool(name="wtpool", bufs=2))
            opool = ctx.enter_context(tc.tile_pool(name="opool", bufs=6))
            pspool = ctx.enter_context(
                tc.tile_pool(name="pspool", bufs=6, space="PSUM")
            )

            AT_sb = resident.tile([128, KO * R], BF)
            nc.sync.dma_start(out=AT_sb, in_=ATp.ap())
            AT_r = AT_sb.rearrange("p (ko r) -> p ko r", r=R)
            WB_sb = resident.tile([128, D_OUT], BF)
            nc.sync.dma_start(out=WB_sb, in_=WBaug.ap())

            # xa^T augmented: rows 0-15 = 2*A@x^T, row 16 = ones, rest zero
            xaT_sb = resident.tile([128, T], BF)
            nc.any.memset(xaT_sb, 0.0)
            nc.sync.dma_start(out=xaT_sb[R : R + 1, :], in_=ones.ap())

            for mb in range(MB):
                # Resident x^T block: [128, 32, TB] bf16
                xT_sb = xpool.tile([128, KO, TB], BF, tag="xTblk")
                for ko in range(KO):
                    nc.sync.dma_start(
                        out=xT_sb[:, ko, :],
                        in_=xT_r[:, ko, ts(mb, TB)],
                    )

                # Prologue: xa^T[r, t] = sum_k (2A)^T[k, r] * x^T[k, t]
                for tw in range(TB // 512):
                    ps_xa = pspool.tile([16, 512], F32, tag="ps_xa", bufs=2)
                    for ko in range(KO):
                        nc.tensor.matmul(
                            ps_xa,
                            AT_r[:, ko, :],
                            xT_sb[:, ko, ts(tw, 512)],
                            start=(ko == 0),
                            stop=(ko == KO - 1),
                        )
                    nc.vector.tensor_copy(
                        out=xaT_sb[0:R, mb * TB + tw * 512 : mb * TB + (tw + 1) * 512],
                        in_=ps_xa,
                    )

                # Main: out[m-tile, n-tile] = sum_ko xT_k^T @ WT_k  (+ aug)
                wt_cached = None
                for n in range(N_TILES):
                    if NOWT and wt_cached is not None:
                        wt_sb = wt_cached
                    else:
                        wt_sb = wtpool.tile([128, KO, NT], BF, tag="wt")
                        for kh in range(2):
                            nc.sync.dma_start(
                                out=wt_sb[:, ts(kh, KO // 2), :],
                                in_=WT_r[:, ts(kh, KO // 2), ts(n, NT)],
                            )
                        wt_cached = wt_sb
                    for m in range(M_TILES):
                        ps = pspool.tile([128, NT], F32, tag="ps")
                        for ko in range(0, KO, k_stride):
                            nc.tensor.matmul(
                                ps,
                                xT_sb[:, ko, ts(m, 128)],
                                wt_sb[:, ko, :],
                                start=(ko == 0),
                                stop=False,
                            )
                        gm = mb * M_TILES + m  # global m-tile
                        nc.tensor.matmul(
                            ps,
                            xaT_sb[:, ts(gm, 128)],
                            WB_sb[:, ts(n, NT)],
                            start=False,
                            stop=True,
                        )
                        if not NOCOPY:
                            ob = opool.tile([128, NT], F32, tag="ob")
                            nc.vector.tensor_copy(out=ob, in_=ps)
                            if not NOSTORE:
                                nc.scalar.dma_start(
                                    out=out_ap[ts(gm, 128), ts(n, NT)], in_=ob
                                )

    nc.compile()
    return nc


def _get_nc(variant=None):
    key = "nc_" + (variant or VARIANT)
    if key not in _CACHE:
        _CACHE[key] = _build_bass(variant)
    return _CACHE[key]


def _prep_inputs(x, W, b, A, B):
    xf = np.ascontiguousarray(x.reshape(TOK, D_IN)).astype(BF16)
    WTh = np.ascontiguousarray(W.T).astype(BF16)            # [d_in, d_out]
    ATh = (SCALING * A).T.astype(BF16)                       # [d_in, r]
    ATp = np.ascontiguousarray(
        ATh.reshape(KO, 128, R).transpose(1, 0, 2).reshape(128, KO * R)
    )
    WBh = np.zeros((128, D_OUT), dtype=BF16)
    WBh[0:R] = B.T.astype(BF16)
    WBh[R] = b.astype(BF16)

    in_maps = []
    for c in range(N_CORES):
        xTc = np.ascontiguousarray(xf[c * T : (c + 1) * T].T)  # [d_in, T] bf16
        in_maps.append(
            {"xT": xTc, "WT": WTh, "ATp": ATp, "WBaug": WBh, "ones": _ONES}
        )
    return in_maps


def kernel(x, W, b, A, B):
    from concourse.bass_utils import run_bass_kernel_spmd

    nc = _get_nc()
    in_maps = _prep_inputs(x, W, b, A, B)
    res = run_bass_kernel_spmd(nc, in_maps, core_ids=list(range(N_CORES)))
    outs = [r["out"] for r in res.results]
    return np.concatenate(outs, axis=0).reshape(B_, S, D_OUT).astype(np.float32)
